# revision 1
# baseline (speedup 1.0000x reference)
# Trainium2 Bass kernel for CoAttentionModule (axial co-attention, 8 heads).
#
# Sharding: data-parallel over (direction, batch) = 2 x 4 = 8 NeuronCores.
# Core c computes weighted = _coattention(qf, rf)[b].T for its (d, b) pair;
# the host concatenates [features, weighted] per direction.
#
# On-chip layout: the hw axis is w-major everywhere (col = w*32 + i, i = h
# index); the host pre-permutes features and un-permutes the output. This
# makes every matmul stationary operand a contiguous SBUF slice (walrus
# requires single-free-dim weight APs).
#
# Per-core pipeline (bf16 matmul operands, fp32 PSUM accumulation):
#   qT = Wq.T @ xq (+bq)          [c_out, hw]
#   kT = Wk.T @ xr  + RWF         RWF[c,(w,k)] = rel_emb[(k-w)%63, c]  (rel_w
#                                 term folded into keys; bk cancels in softmax)
#   v  = xr.T @ Wv                [(w,k), c]
#   QAUG[t', col(w,i)] = sum_c rel_emb[(t'-i)%63, c] q[c, col]  (63 aug rows,
#                                 via 32 host-rolled copies of rel_emb.T)
#   scores tile (head n, w-group of 4) [128=(w,i), 128=(w,k)]:
#       q.k' + QAUG.KAUG(one-hot) + WIND.KMASK(-1e30 off-diag mask channels)
#   softmax: exp(scale=1/16) with accum_out row sums -> reciprocal -> scale
#   probsT via DVE 32x32 stream transpose (block-diagonal => exact transpose)
#   avT[c,(w,i)] = v.T @ probsT ; output proj outT = Wo.T @ attT + bo'
#   (bv folded on host: bo' = bv @ Wo + bo; bk dropped: softmax-invariant)
import numpy as np
import ml_dtypes

B, C, H, W = 4, 2048, 32, 32
HW = H * W
NH, HD = 8, 256
T = 2 * max(H, W) - 1  # 63
NC = C // 128  # 16 chunks

_CACHE = {}


def _hostprep(Wq, bq, Wk, bk, Wv, bv, Wo, bo, rel_emb):
    bf = ml_dtypes.bfloat16
    f32 = np.float32
    Wq, Wk, Wv, Wo = (np.asarray(a, f32) for a in (Wq, Wk, Wv, Wo))
    rel = np.asarray(rel_emb, f32)  # [63, 256]
    ii = np.arange(32)

    # lhsT blobs [co, p, ci*128+m]: one contiguous [128, 2048] DMA per co chunk
    def lchunks(Wm):
        return np.ascontiguousarray(
            Wm.reshape(NC, 128, NC, 128).transpose(2, 1, 0, 3).reshape(NC, 128, C)
        ).astype(bf)

    wq_l = lchunks(Wq)
    wk_l = lchunks(Wk)
    wo_l = lchunks(Wo)
    # V weights per head [n, p, ci*256+m]: one contiguous [128, 4096] DMA per head
    wv_r = np.ascontiguousarray(
        Wv.reshape(NC, 128, NH, HD).transpose(2, 1, 0, 3).reshape(NH, 128, NC * HD)
    ).astype(bf)

    bq_c = np.ascontiguousarray(np.asarray(bq, f32).reshape(NC, 128).T)  # [128,16]
    bo2 = np.asarray(bv, f32) @ Wo + np.asarray(bo, f32)
    bo2_c = np.ascontiguousarray(bo2.reshape(NC, 128).T)  # [128,16]

    w_idx, k_idx = np.meshgrid(np.arange(32), np.arange(32), indexing="ij")
    # rel_w fold table, w-major [2, 128, 1024]: rwf[ch, p, w*32+k] = rel[(k-w)%63, ch*128+p]
    rwf = rel[(k_idx - w_idx) % T].reshape(HW, HD)  # [(w,k), 256]
    rwf = np.ascontiguousarray(rwf.T.reshape(2, 128, HW)).astype(f32)
    # rolled rel_emb.T for QAUG: relroll[p, (i, ch, t')] = rel[(t'-i)%63, ch*128+p]
    # t' padded 63->128 with zeros so the stationary operand is 128 wide (FWL)
    relroll = np.zeros((128, 32 * 2 * 128), f32)
    for i in range(32):
        for ch in range(2):
            blk = rel[(np.arange(T) - i) % T, ch * 128:(ch + 1) * 128]  # [63,128]
            relroll[:, (i * 2 + ch) * 128:(i * 2 + ch) * 128 + T] = blk.T
    relroll = relroll.astype(bf)
    # key-side aug channels [96, 1024] w-major: rows 0:63 one-hot rel gather
    # (kaug[t, w*32+k] = t==k), row 63 zero, rows 64:96 block-diag mask
    # (kmask[w', w*32+k] = 0 if w==w' else -1e30). Query side: rows 0:63 QAUG,
    # row 63 zero, rows 64:96 w-indicator.
    kaug = np.zeros((96, HW), f32)
    kaug[k_idx.reshape(-1), np.arange(HW)] = 1.0
    kaug[64:96] = -1e30
    wind = np.zeros((32, HW), f32)
    for w in range(32):
        wind[w, w * 32 + ii] = 1.0  # query col w*32+i
        kaug[64 + w, w * 32 + ii] = 0.0  # key col w*32+k
    kaug = kaug.astype(bf)
    wind = wind.astype(bf)

    return dict(wq_l=wq_l, wk_l=wk_l, wo_l=wo_l, wv_r=wv_r, bq_c=bq_c,
                bo2_c=bo2_c, rwf=rwf, relroll=relroll, kaug=kaug, wind=wind)


def _build(timing_twin=False, loop=1):
    import concourse.bacc as bacc
    import concourse.mybir as mybir
    import concourse.tile as tile

    F32, BF16 = mybir.dt.float32, mybir.dt.bfloat16
    nc = bacc.Bacc(None, target_bir_lowering=False)

    if timing_twin:
        # timing-equivalent NEFF: big tensors live in internal DRAM scratch
        # (no per-call host staging), only a tiny external in/out pair.
        def declare(name, shape, dt, isOutput=False):
            return nc.dram_tensor(name, shape, dt)
        tiny_in = nc.declare_dram_parameter("tiny_in", [1, 4], F32, isOutput=False)
        tiny_out = nc.declare_dram_parameter("tiny_out", [1, 4], F32, isOutput=True)
    else:
        declare = nc.declare_dram_parameter

    xq = declare("xq", [C, HW], BF16, isOutput=False)
    xr = declare("xr", [C, HW], BF16, isOutput=False)
    wq_l = declare("wq_l", [NC, 128, C], BF16, isOutput=False)
    wk_l = declare("wk_l", [NC, 128, C], BF16, isOutput=False)
    wo_l = declare("wo_l", [NC, 128, C], BF16, isOutput=False)
    wv_r = declare("wv_r", [NH, 128, NC * HD], BF16, isOutput=False)
    bq_c = declare("bq_c", [128, NC], F32, isOutput=False)
    bo2_c = declare("bo2_c", [128, NC], F32, isOutput=False)
    rwf = declare("rwf", [2, 128, HW], F32, isOutput=False)
    relroll = declare("relroll", [128, 32 * 2 * 128], BF16, isOutput=False)
    kaug = declare("kaug", [96, HW], BF16, isOutput=False)
    wind = declare("wind", [32, HW], BF16, isOutput=False)
    out = declare("out", [C, HW], F32, isOutput=True)

    EXP = mybir.ActivationFunctionType.Exp

    with tile.TileContext(nc) as tc:
        with (
            tc.tile_pool(name="feat", bufs=2) as feat_pool,
            tc.tile_pool(name="att", bufs=1) as att_pool,
            tc.tile_pool(name="const", bufs=1) as const_pool,
            tc.tile_pool(name="head", bufs=2) as head_pool,
            tc.tile_pool(name="wstr", bufs=3) as wstr_pool,
            tc.tile_pool(name="probs", bufs=3) as probs_pool,
            tc.tile_pool(name="outs", bufs=3) as outs_pool,
            tc.tile_pool(name="psum", bufs=3, space="PSUM") as psum_pool,
            tc.tile_pool(name="psumb", bufs=4, space="PSUM") as psumb_pool,
            tc.tile_pool(name="psumq", bufs=1, space="PSUM") as psumq_pool,
        ):
            # ---- load features + constants (resident) ----
            # xq first (gates the very first Q-proj groups), then xr, then
            # constants (needed later) so the PE cold-start wait is minimal.
            xqt = feat_pool.tile([128, NC * HW], BF16, tag="feat")
            xrt = feat_pool.tile([128, NC * HW], BF16, tag="feat")
            for cc in range(NC):
                nc.sync.dma_start(xqt[:, cc * HW:(cc + 1) * HW], xq[cc * 128:(cc + 1) * 128, :])
            for cc in range(NC):
                nc.sync.dma_start(xrt[:, cc * HW:(cc + 1) * HW], xr[cc * 128:(cc + 1) * 128, :])
            attT = att_pool.tile([128, NC * HW], BF16)

            c_kaug = const_pool.tile([96, HW], BF16)
            nc.sync.dma_start(c_kaug[:], kaug[:])
            c_wind = const_pool.tile([32, HW], BF16)
            nc.sync.dma_start(c_wind[:], wind[:])
            c_rwf = const_pool.tile([128, 2 * HW], F32)
            nc.sync.dma_start(c_rwf[:, 0:HW], rwf[0])
            nc.sync.dma_start(c_rwf[:, HW:2 * HW], rwf[1])
            c_roll = const_pool.tile([128, 32 * 2 * 128], BF16)
            nc.sync.dma_start(c_roll[:], relroll[:])
            c_bq = const_pool.tile([128, NC], F32)
            nc.sync.dma_start(c_bq[:], bq_c[:])
            c_bo = const_pool.tile([128, NC], F32)
            nc.sync.dma_start(c_bo[:], bo2_c[:])

            for rep in range(loop):
                for n in range(NH):
                    sq = head_pool.tile([128, 2 * HW], BF16, tag="sq")
                    sk = head_pool.tile([128, 2 * HW], BF16, tag="sk")
                    sv = head_pool.tile([128, NH * HD], BF16, tag="sv")
                    sqa = head_pool.tile([96, HW], BF16, tag="sqa")
                    swv = head_pool.tile([128, NC * HD], BF16, tag="swv")

                    # stage this head's V weights once (one 1MB DMA)
                    nc.sync.dma_start(swv[:], wv_r[n])
                    # aug rows 32:96: zero row 63, w-indicator rows 64:96
                    nc.vector.memset(sqa[:], 0.0)
                    nc.vector.tensor_copy(sqa[64:96, :], c_wind[:])

                    # ---- Q / K projections: psum[co2,h2] = sum_ci W.T @ x ----
                    for which in range(2):  # 0 = Q, 1 = K
                        wsrc = wq_l if which == 0 else wk_l
                        xsrc = xqt if which == 0 else xrt
                        dst = sq if which == 0 else sk
                        for co2 in range(2):
                            co = n * 2 + co2
                            wt = wstr_pool.tile([128, C], BF16, tag="wl")
                            nc.sync.dma_start(wt[:], wsrc[co])
                            for h2 in range(2):
                                ps = psum_pool.tile([128, 512], F32, tag="pp")
                                for ci in range(NC):
                                    nc.tensor.matmul(
                                        ps[:], wt[:, ci * 128:(ci + 1) * 128],
                                        xsrc[:, ci * HW + h2 * 512: ci * HW + (h2 + 1) * 512],
                                        start=(ci == 0), stop=(ci == NC - 1))
                                dpos = dst[:, co2 * HW + h2 * 512: co2 * HW + (h2 + 1) * 512]
                                if which == 0:
                                    nc.vector.tensor_scalar_add(dpos, ps[:], c_bq[:, co:co + 1])
                                else:
                                    nc.vector.tensor_add(
                                        dpos, ps[:],
                                        c_rwf[:, co2 * HW + h2 * 512: co2 * HW + (h2 + 1) * 512])

                    # ---- V projection, w-major rows ----
                    for wg in range(8):
                        psv = psum_pool.tile([128, HD], F32, tag="pp")
                        for ci in range(NC):
                            nc.tensor.matmul(
                                psv[:], xrt[:, ci * HW + wg * 128: ci * HW + (wg + 1) * 128],
                                swv[:, ci * HD:(ci + 1) * HD],
                                start=(ci == 0), stop=(ci == NC - 1))
                        nc.vector.tensor_copy(sv[:, wg * HD:(wg + 1) * HD], psv[:])

                    # ---- QAUG: per query-row i, rolled rel_emb.T contraction ----
                    for half in range(2):
                        pqa = psumq_pool.tile([128, 512], F32, tag="qa")
                        for io in range(16):
                            i = half * 16 + io
                            for ch in range(2):
                                nc.tensor.matmul(
                                    pqa[:, io * 32:(io + 1) * 32],
                                    c_roll[:, (i * 2 + ch) * 128:(i * 2 + ch + 1) * 128],
                                    sq[:, ch * HW + i: (ch + 1) * HW: 32],
                                    start=(ch == 0), stop=(ch == 1))
                        # pqa cols are (i, w) pairs; sqa is w-major (w*32+i)
                        nc.vector.tensor_copy(
                            sqa[0:T, :].rearrange("p (w i) -> p i w", i=32)[:, half * 16:(half + 1) * 16, :],
                            pqa[0:T, :].rearrange("p (i w) -> p i w", w=32))

                    # ---- attention per w-group ----
                    for wg in range(8):
                        sc = psumb_pool.tile([128, 128], F32, tag="sa")
                        nc.tensor.matmul(sc[:], sq[:, wg * 128:(wg + 1) * 128],
                                         sk[:, wg * 128:(wg + 1) * 128],
                                         start=True, stop=False)
                        nc.tensor.matmul(sc[:], sq[:, HW + wg * 128: HW + (wg + 1) * 128],
                                         sk[:, HW + wg * 128: HW + (wg + 1) * 128],
                                         start=False, stop=False)
                        nc.tensor.matmul(sc[:], sqa[:, wg * 128:(wg + 1) * 128],
                                         c_kaug[:, wg * 128:(wg + 1) * 128],
                                         start=False, stop=True)
                        probs = probs_pool.tile([128, 128], BF16, tag="pr")
                        sums = probs_pool.tile([128, 1], F32, tag="sm")
                        recip = probs_pool.tile([128, 1], F32, tag="rc")
                        nc.scalar.activation(probs[:], sc[:], EXP, scale=1.0 / 16.0,
                                             accum_out=sums[:])
                        nc.vector.reciprocal(recip[:], sums[:])
                        nc.vector.tensor_scalar_mul(probs[:], probs[:], recip[:])
                        probsT = probs_pool.tile([128, 128], BF16, tag="prT")
                        nc.vector.transpose(probsT[:], probs[:])
                        for ch in range(2):
                            av = psumb_pool.tile([128, 128], F32, tag="sa")
                            nc.tensor.matmul(
                                av[:], sv[:, wg * HD + ch * 128: wg * HD + (ch + 1) * 128],
                                probsT[:], start=True, stop=True)
                            nc.vector.tensor_copy(
                                attT[:, (n * 2 + ch) * HW + wg * 128:
                                     (n * 2 + ch) * HW + (wg + 1) * 128], av[:])

                # ---- output projection ----
                for co in range(NC):
                    wt = wstr_pool.tile([128, C], BF16, tag="wl")
                    nc.sync.dma_start(wt[:], wo_l[co])
                    for h2 in range(2):
                        ps = psum_pool.tile([128, 512], F32, tag="pp")
                        for ci in range(NC):
                            nc.tensor.matmul(
                                ps[:], wt[:, ci * 128:(ci + 1) * 128],
                                attT[:, ci * HW + h2 * 512: ci * HW + (h2 + 1) * 512],
                                start=(ci == 0), stop=(ci == NC - 1))
                        ot = outs_pool.tile([128, 512], F32, tag="ot")
                        nc.vector.tensor_scalar_add(ot[:], ps[:], c_bo[:, co:co + 1])
                        nc.sync.dma_start(
                            out[co * 128:(co + 1) * 128, h2 * 512:(h2 + 1) * 512], ot[:])

                if timing_twin:
                    tt = outs_pool.tile([1, 4], F32, tag="tt")
                    nc.sync.dma_start(tt[:], tiny_in[:])
                    nc.sync.dma_start(tiny_out[:], tt[:])

            if timing_twin:
                tt = outs_pool.tile([1, 4], F32, tag="tt")
                nc.sync.dma_start(tt[:], tiny_in[:])
                nc.sync.dma_start(tiny_out[:], tt[:])

    nc.finalize()
    return nc


def kernel(left_features, right_features, Wq, bq, Wk, bk, Wv, bv, Wo, bo, rel_emb,
           _trace=False):
    from concourse.bass_utils import run_bass_kernel_spmd

    bf = ml_dtypes.bfloat16
    if "nc" not in _CACHE:
        _CACHE["nc"] = _build()
    nc = _CACHE["nc"]

    consts = _hostprep(Wq, bq, Wk, bk, Wv, bv, Wo, bo, rel_emb)
    lf = np.asarray(left_features, np.float32)
    rf = np.asarray(right_features, np.float32)

    def wmajor(x):  # (C, H, W) -> (C, HW) with col = w*32 + i
        return np.ascontiguousarray(x.transpose(0, 2, 1).reshape(C, HW)).astype(bf)

    in_maps = []
    for core in range(8):
        d, b = divmod(core, 4)
        qf = lf[b] if d == 0 else rf[b]
        rfb = rf[b] if d == 0 else lf[b]
        m = dict(consts)
        m["xq"] = wmajor(qf)
        m["xr"] = wmajor(rfb)
        in_maps.append(m)

    res = run_bass_kernel_spmd(nc, in_maps, list(range(8)), trace=_trace)
    _CACHE["last_result"] = res

    def unperm(o):  # [C, HW w-major] -> (C, H, W)
        return np.ascontiguousarray(o.reshape(C, W, H).transpose(0, 2, 1))

    wr = np.stack([unperm(res.results[b]["out"]) for b in range(4)])
    wl = np.stack([unperm(res.results[4 + b]["out"]) for b in range(4)])
    left_att = np.concatenate([lf, wr], axis=1)
    right_att = np.concatenate([rf, wl], axis=1)
    return (left_att, right_att)



# revision 10
# speedup vs baseline: 2.9728x; 2.9728x over previous
# Trainium2 Bass kernel for CoAttentionModule (axial co-attention, 8 heads).
#
# Sharding: data-parallel over (direction, batch) = 2 x 4 = 8 NeuronCores.
# Core c computes weighted = _coattention(qf, rf)[b].T for its (d, b) pair;
# the host concatenates [features, weighted] per direction.
#
# On-chip layout: the hw axis is w-major everywhere (col = w*32 + i, i = h
# index); the host pre-permutes features and un-permutes the output. This
# makes every matmul stationary operand a contiguous SBUF slice.
#
# The Q and K projections run in fp8e4m3 with DoubleRow perf
# mode (contracts 256 channels per pass = 2x bf16 throughput). Scales keep
# every fp8 operand inside the e4m3 normal range:
#   features x8 = fp8(x * SX), weights W8 = fp8(W * SW)
#   q_s/k_s are scaled by SX*SW; the dequant folds into the softmax scale.
#   v_s scaled by SX*SW; attT8 = av_psum * SA/(SX*SW)  (att * SA, SA=16)
#   out = psum / (SA*SW) + bias
# Attention internals (scores, softmax, AV) stay bf16/fp32 as in the bf16
# kernel:
#   qT = Wq.T @ xq (+bq)          [c_out, hw]
#   kT = Wk.T @ xr  + RWF         RWF[c,(w,k)] = rel[(k-w)%63, c]*SX*SW (rel_w
#                                 folded into keys; bk cancels in softmax)
#   v  = xr.T @ Wv                [(w,k), c]
#   QAUG[t', col(w,i)] = sum_c relroll q  (63 aug rows, rolled rel_emb.T)
#   scores tile (head n, w-group of 4) [128=(w,i), 128=(w,k)]:
#       q.k' + QAUG.KAUG(one-hot) + WIND.KMASK(-1e30 off-diag mask channels)
#   softmax: exp(scale=1/(16*(SX*SW)^2)) with accum_out row sums -> recip
#   probsT via DVE 32x32 stream transpose (block-diagonal => exact transpose)
#   avT[c,(w,i)] = v.T @ probsT ; output proj outT = Wo.T @ attT + bo'
#   (bv folded on host: bo' = bv @ Wo + bo; bk dropped: softmax-invariant)
import numpy as np
import ml_dtypes

B, C, H, W = 4, 2048, 32, 32
HW = H * W
NH, HD = 8, 256
T = 2 * max(H, W) - 1  # 63
NC = C // 128  # 16 chunks

SX = 8.0     # feature fp8 scale
SW = 512.0   # weight fp8 scale
SA = 16.0    # attT fp8 scale (att values ~N(0, 0.4^2), absmax ~5)

QK_FP8 = True
V_FP8 = False
O_FP8 = False

_CACHE = {}


def _hostprep(Wq, bq, Wk, bk, Wv, bv, Wo, bo, rel_emb):
    bf = ml_dtypes.bfloat16
    f8 = ml_dtypes.float8_e4m3
    f32 = np.float32
    Wq, Wk, Wv, Wo = (np.asarray(a, f32) for a in (Wq, Wk, Wv, Wo))
    rel = np.asarray(rel_emb, f32)  # [63, 256]
    ii = np.arange(32)
    sqs = SX * SW if QK_FP8 else 1.0  # scale of q_s and k_s

    # lhsT blobs [co, p, ci*128+m]: one contiguous [128, 2048] DMA per co chunk
    def lchunks(Wm, dt, s):
        return np.ascontiguousarray(
            (Wm * s).reshape(NC, 128, NC, 128).transpose(2, 1, 0, 3).reshape(NC, 128, C)
        ).astype(dt)

    wq_l = lchunks(Wq, f8 if QK_FP8 else bf, SW if QK_FP8 else 1.0)
    wk_l = lchunks(Wk, f8 if QK_FP8 else bf, SW if QK_FP8 else 1.0)
    wo_l = lchunks(Wo, f8 if O_FP8 else bf, SW if O_FP8 else 1.0)
    # V weights per head [n, p, ci*256+m]: one contiguous [128, 4096] DMA per head
    wv_r = np.ascontiguousarray(
        (Wv * (SW if V_FP8 else 1.0))
        .reshape(NC, 128, NH, HD).transpose(2, 1, 0, 3).reshape(NH, 128, NC * HD)
    ).astype(f8 if V_FP8 else bf)

    bq_c = np.ascontiguousarray((np.asarray(bq, f32) * sqs).reshape(NC, 128).T)
    bo2 = np.asarray(bv, f32) @ Wo + np.asarray(bo, f32)
    bo2_c = np.ascontiguousarray(bo2.reshape(NC, 128).T)  # [128,16]

    w_idx, k_idx = np.meshgrid(np.arange(32), np.arange(32), indexing="ij")
    # rel_w fold table, w-major [2, 128, 1024]: rwf[ch, p, w*32+k] = rel[(k-w)%63, ch*128+p]
    rwf = (rel * sqs)[(k_idx - w_idx) % T].reshape(HW, HD)  # [(w,k), 256]
    rwf = np.ascontiguousarray(rwf.T.reshape(2, 128, HW)).astype(bf)
    # rolled rel_emb.T for QAUG: relroll[p, (i, ch, t')] = rel[(t'-i)%63, ch*128+p]
    # t' padded 63->128 with zeros so the stationary operand is 128 wide (FWL)
    relroll = np.zeros((128, 32 * 2 * 128), f32)
    for i in range(32):
        for ch in range(2):
            blk = (rel * sqs)[(np.arange(T) - i) % T, ch * 128:(ch + 1) * 128]
            relroll[:, (i * 2 + ch) * 128:(i * 2 + ch) * 128 + T] = blk.T
    relroll = relroll.astype(bf)
    # key-side aug channels [96, 1024] w-major: rows 0:63 one-hot rel gather
    # (kaug[t, w*32+k] = t==k), row 63 zero, rows 64:96 block-diag mask
    # (kmask[w', w*32+k] = 0 if w==w' else -1e30). Query side: rows 0:63 QAUG,
    # row 63 zero, rows 64:96 w-indicator.
    kaug = np.zeros((96, HW), f32)
    kaug[k_idx.reshape(-1), np.arange(HW)] = 1.0
    kaug[64:96] = -1e30
    wind = np.zeros((32, HW), f32)
    for w in range(32):
        wind[w, w * 32 + ii] = 1.0  # query col w*32+i
        kaug[64 + w, w * 32 + ii] = 0.0  # key col w*32+k
    kaug = kaug.astype(bf)
    wind = wind.astype(bf)

    return dict(wq_l=wq_l, wk_l=wk_l, wo_l=wo_l, wv_r=wv_r, bq_c=bq_c,
                bo2_c=bo2_c, rwf=rwf, relroll=relroll, kaug=kaug, wind=wind)


def _build(timing_twin=False, loop=1):
    import concourse.bacc as bacc
    import concourse.mybir as mybir
    import concourse.tile as tile

    F32, BF16, F8 = mybir.dt.float32, mybir.dt.bfloat16, mybir.dt.float8e4
    DR = mybir.MatmulPerfMode.DoubleRow
    QKDT = F8 if QK_FP8 else BF16
    VDT = F8 if V_FP8 else BF16
    ODT = F8 if O_FP8 else BF16
    sqs = SX * SW if QK_FP8 else 1.0
    svs = SX * SW if V_FP8 else 1.0
    EXPSCALE = float(1.0 / (16.0 * sqs * sqs))
    ATTSCALE = float((SA if O_FP8 else 1.0) / svs)
    ODEQ = float(1.0 / (SA * SW)) if O_FP8 else 1.0
    nc = bacc.Bacc(None, target_bir_lowering=False)

    if timing_twin:
        # timing-equivalent NEFF: big tensors live in internal DRAM scratch
        # (no per-call host staging), only a tiny external in/out pair.
        def declare(name, shape, dt, isOutput=False):
            return nc.dram_tensor(name, shape, dt)
        tiny_in = nc.declare_dram_parameter("tiny_in", [1, 4], F32, isOutput=False)
        tiny_out = nc.declare_dram_parameter("tiny_out", [1, 4], F32, isOutput=True)
    else:
        declare = nc.declare_dram_parameter

    xq = declare("xq", [C, HW], QKDT, isOutput=False)
    xr = declare("xr", [C, HW], QKDT, isOutput=False)
    xrv = declare("xrv", [C, HW], VDT, isOutput=False) if V_FP8 != QK_FP8 else None
    wq_l = declare("wq_l", [NC, 128, C], QKDT, isOutput=False)
    wk_l = declare("wk_l", [NC, 128, C], QKDT, isOutput=False)
    wo_l = declare("wo_l", [NC, 128, C], ODT, isOutput=False)
    wv_r = declare("wv_r", [NH, 128, NC * HD], VDT, isOutput=False)
    bq_c = declare("bq_c", [128, NC], F32, isOutput=False)
    bo2_c = declare("bo2_c", [128, NC], F32, isOutput=False)
    rwf = declare("rwf", [2, 128, HW], BF16, isOutput=False)
    relroll = declare("relroll", [128, 32 * 2 * 128], BF16, isOutput=False)
    kaug = declare("kaug", [96, HW], BF16, isOutput=False)
    wind = declare("wind", [32, HW], BF16, isOutput=False)
    out = declare("out", [C, HW], F32, isOutput=True)

    EXP = mybir.ActivationFunctionType.Exp

    with tile.TileContext(nc) as tc:
        with (
            tc.tile_pool(name="feat", bufs=2) as feat_pool,
            tc.tile_pool(name="att", bufs=1) as att_pool,
            tc.tile_pool(name="const", bufs=1) as const_pool,
            tc.tile_pool(name="head", bufs=2) as head_pool,
            tc.tile_pool(name="wstr", bufs=3) as wstr_pool,
            tc.tile_pool(name="probs", bufs=3) as probs_pool,
            tc.tile_pool(name="outs", bufs=3) as outs_pool,
            tc.tile_pool(name="psum", bufs=3, space="PSUM") as psum_pool,
            tc.tile_pool(name="psumb", bufs=4, space="PSUM") as psumb_pool,
            tc.tile_pool(name="psumq", bufs=1, space="PSUM") as psumq_pool,
        ):
            # ---- load features + constants (resident) ----
            # xq first (gates the very first Q-proj groups), then xr, then
            # constants (needed later) so the PE cold-start wait is minimal.
            xqt = feat_pool.tile([128, NC * HW], QKDT, tag="feat")
            xrt = feat_pool.tile([128, NC * HW], QKDT, tag="feat")
            for cc in range(NC):
                nc.sync.dma_start(xqt[:, cc * HW:(cc + 1) * HW], xq[cc * 128:(cc + 1) * 128, :])
            for cc in range(NC):
                nc.sync.dma_start(xrt[:, cc * HW:(cc + 1) * HW], xr[cc * 128:(cc + 1) * 128, :])
            if xrv is not None:
                xrvt = feat_pool.tile([128, NC * HW], VDT, tag="featv", bufs=1)
                for cc in range(NC):
                    nc.sync.dma_start(xrvt[:, cc * HW:(cc + 1) * HW], xrv[cc * 128:(cc + 1) * 128, :])
            else:
                xrvt = xrt
            attT = att_pool.tile([128, NC * HW], ODT)

            c_kaug = const_pool.tile([96, HW], BF16)
            nc.sync.dma_start(c_kaug[:], kaug[:])
            c_wind = const_pool.tile([32, HW], BF16)
            nc.sync.dma_start(c_wind[:], wind[:])
            c_rwf = const_pool.tile([128, 2 * HW], BF16)
            nc.sync.dma_start(c_rwf[:, 0:HW], rwf[0])
            nc.sync.dma_start(c_rwf[:, HW:2 * HW], rwf[1])
            c_roll = const_pool.tile([128, 32 * 2 * 128], BF16)
            nc.sync.dma_start(c_roll[:], relroll[:])
            c_bq = const_pool.tile([128, NC], F32)
            nc.sync.dma_start(c_bq[:], bq_c[:])
            c_bo = const_pool.tile([128, NC], F32)
            nc.sync.dma_start(c_bo[:], bo2_c[:])

            x3q = xqt.rearrange("p (c w) -> p c w", c=NC)
            x3r = xrt.rearrange("p (c w) -> p c w", c=NC)
            x3rv = xrvt.rearrange("p (c w) -> p c w", c=NC)
            att3 = attT.rearrange("p (c w) -> p c w", c=NC)

            for rep in range(loop):
                for n in range(NH):
                    sq = head_pool.tile([128, 2 * HW], BF16, tag="sq")
                    sk = head_pool.tile([128, 2 * HW], BF16, tag="sk")
                    sv = head_pool.tile([128, NH * HD], BF16, tag="sv")
                    sqa = head_pool.tile([96, HW], BF16, tag="sqa")
                    swv = head_pool.tile([128, NC * HD], VDT, tag="swv")

                    # stage this head's V weights once (one DMA)
                    nc.sync.dma_start(swv[:], wv_r[n])
                    # aug rows 32:96: zero row 63, w-indicator rows 64:96
                    nc.vector.memset(sqa[:], 0.0)
                    nc.vector.tensor_copy(sqa[64:96, :], c_wind[:])

                    # ---- Q / K projections: psum[co2, cols] = W.T @ x ----
                    for which in range(2):  # 0 = Q, 1 = K
                        wsrc = wq_l if which == 0 else wk_l
                        xsrc = x3q if which == 0 else x3r
                        dst = sq if which == 0 else sk
                        for co2 in range(2):
                            co = n * 2 + co2
                            wt = wstr_pool.tile([128, C], QKDT, tag="wl")
                            nc.sync.dma_start(wt[:], wsrc[co])
                            w3 = wt.rearrange("p (c m) -> p c m", c=NC)
                            if QK_FP8:
                                for g in range(4):
                                    ps = psum_pool.tile([128, 256], F32, tag="pp")
                                    for c2 in range(8):
                                        nc.tensor.matmul(
                                            ps[:], w3[:, 2 * c2:2 * c2 + 2, :],
                                            xsrc[:, 2 * c2:2 * c2 + 2, g * 256:(g + 1) * 256],
                                            start=(c2 == 0), stop=(c2 == 7),
                                            perf_mode=DR)
                                    dpos = dst[:, co2 * HW + g * 256: co2 * HW + (g + 1) * 256]
                                    if which == 0:
                                        nc.vector.tensor_scalar_add(dpos, ps[:], c_bq[:, co:co + 1])
                                    else:
                                        nc.vector.tensor_add(
                                            dpos, ps[:],
                                            c_rwf[:, co2 * HW + g * 256: co2 * HW + (g + 1) * 256])
                            else:
                                for h2 in range(2):
                                    ps = psum_pool.tile([128, 512], F32, tag="pp")
                                    for ci in range(NC):
                                        nc.tensor.matmul(
                                            ps[:], w3[:, ci, :],
                                            xsrc[:, ci, h2 * 512:(h2 + 1) * 512],
                                            start=(ci == 0), stop=(ci == NC - 1))
                                    dpos = dst[:, co2 * HW + h2 * 512: co2 * HW + (h2 + 1) * 512]
                                    if which == 0:
                                        nc.vector.tensor_scalar_add(dpos, ps[:], c_bq[:, co:co + 1])
                                    else:
                                        nc.vector.tensor_add(
                                            dpos, ps[:],
                                            c_rwf[:, co2 * HW + h2 * 512: co2 * HW + (h2 + 1) * 512])

                    # ---- V projection, w-major rows ----
                    w3v = swv.rearrange("p (c m) -> p c m", c=NC)
                    for wg in range(8):
                        psv = psum_pool.tile([128, 256], F32, tag="pp")
                        if V_FP8:
                            for c2 in range(8):
                                nc.tensor.matmul(
                                    psv[:],
                                    x3rv[:, 2 * c2:2 * c2 + 2, wg * 128:(wg + 1) * 128],
                                    w3v[:, 2 * c2:2 * c2 + 2, :],
                                    start=(c2 == 0), stop=(c2 == 7),
                                    perf_mode=DR)
                        else:
                            for ci in range(NC):
                                nc.tensor.matmul(
                                    psv[:], x3rv[:, ci, wg * 128:(wg + 1) * 128],
                                    w3v[:, ci, :],
                                    start=(ci == 0), stop=(ci == NC - 1))
                        nc.vector.tensor_copy(sv[:, wg * HD:(wg + 1) * HD], psv[:])

                    # ---- QAUG: per query-row i, rolled rel_emb.T contraction ----
                    for half in range(2):
                        pqa = psumq_pool.tile([128, 512], F32, tag="qa")
                        for io in range(16):
                            i = half * 16 + io
                            for ch in range(2):
                                nc.tensor.matmul(
                                    pqa[:, io * 32:(io + 1) * 32],
                                    c_roll[:, (i * 2 + ch) * 128:(i * 2 + ch + 1) * 128],
                                    sq[:, ch * HW + i: (ch + 1) * HW: 32],
                                    start=(ch == 0), stop=(ch == 1))
                        # pqa cols are (i, w) pairs; sqa is w-major (w*32+i)
                        nc.vector.tensor_copy(
                            sqa[0:T, :].rearrange("p (w i) -> p i w", i=32)[:, half * 16:(half + 1) * 16, :],
                            pqa[0:T, :].rearrange("p (i w) -> p i w", w=32))

                    # ---- attention per w-group ----
                    for wg in range(8):
                        sc = psumb_pool.tile([128, 128], F32, tag="sa")
                        nc.tensor.matmul(sc[:], sq[:, wg * 128:(wg + 1) * 128],
                                         sk[:, wg * 128:(wg + 1) * 128],
                                         start=True, stop=False)
                        nc.tensor.matmul(sc[:], sq[:, HW + wg * 128: HW + (wg + 1) * 128],
                                         sk[:, HW + wg * 128: HW + (wg + 1) * 128],
                                         start=False, stop=False)
                        nc.tensor.matmul(sc[:], sqa[:, wg * 128:(wg + 1) * 128],
                                         c_kaug[:, wg * 128:(wg + 1) * 128],
                                         start=False, stop=True)
                        probs = probs_pool.tile([128, 128], BF16, tag="pr")
                        sums = probs_pool.tile([128, 1], F32, tag="sm")
                        recip = probs_pool.tile([128, 1], F32, tag="rc")
                        nc.scalar.activation(probs[:], sc[:], EXP, scale=EXPSCALE,
                                             accum_out=sums[:])
                        nc.vector.reciprocal(recip[:], sums[:])
                        nc.vector.tensor_scalar_mul(probs[:], probs[:], recip[:])
                        probsT = probs_pool.tile([128, 128], BF16, tag="prT")
                        nc.vector.transpose(probsT[:], probs[:])
                        for ch in range(2):
                            av = psumb_pool.tile([128, 128], F32, tag="sa")
                            nc.tensor.matmul(
                                av[:], sv[:, wg * HD + ch * 128: wg * HD + (ch + 1) * 128],
                                probsT[:], start=True, stop=True)
                            apos = attT[:, (n * 2 + ch) * HW + wg * 128:
                                        (n * 2 + ch) * HW + (wg + 1) * 128]
                            if ATTSCALE == 1.0:
                                nc.vector.tensor_copy(apos, av[:])
                            else:
                                nc.vector.tensor_scalar_mul(apos, av[:], ATTSCALE)

                # ---- output projection ----
                for co in range(NC):
                    wt = wstr_pool.tile([128, C], ODT, tag="wo")
                    nc.sync.dma_start(wt[:], wo_l[co])
                    w3 = wt.rearrange("p (c m) -> p c m", c=NC)
                    if O_FP8:
                        for g in range(4):
                            ps = psum_pool.tile([128, 256], F32, tag="pp")
                            for c2 in range(8):
                                nc.tensor.matmul(
                                    ps[:], w3[:, 2 * c2:2 * c2 + 2, :],
                                    att3[:, 2 * c2:2 * c2 + 2, g * 256:(g + 1) * 256],
                                    start=(c2 == 0), stop=(c2 == 7),
                                    perf_mode=DR)
                            ot = outs_pool.tile([128, 256], F32, tag="ot")
                            nc.vector.tensor_scalar(
                                ot[:], ps[:], ODEQ, c_bo[:, co:co + 1],
                                mybir.AluOpType.mult, mybir.AluOpType.add)
                            nc.sync.dma_start(
                                out[co * 128:(co + 1) * 128, g * 256:(g + 1) * 256], ot[:])
                    else:
                        for h2 in range(2):
                            ps = psum_pool.tile([128, 512], F32, tag="pp")
                            for ci in range(NC):
                                nc.tensor.matmul(
                                    ps[:], w3[:, ci, :],
                                    att3[:, ci, h2 * 512:(h2 + 1) * 512],
                                    start=(ci == 0), stop=(ci == NC - 1))
                            ot = outs_pool.tile([128, 512], F32, tag="ot")
                            nc.vector.tensor_scalar_add(ot[:], ps[:], c_bo[:, co:co + 1])
                            nc.sync.dma_start(
                                out[co * 128:(co + 1) * 128, h2 * 512:(h2 + 1) * 512], ot[:])

                if timing_twin:
                    tt = outs_pool.tile([1, 4], F32, tag="tt")
                    nc.sync.dma_start(tt[:], tiny_in[:])
                    nc.sync.dma_start(tiny_out[:], tt[:])

            if timing_twin:
                tt = outs_pool.tile([1, 4], F32, tag="tt")
                nc.sync.dma_start(tt[:], tiny_in[:])
                nc.sync.dma_start(tiny_out[:], tt[:])

    nc.finalize()
    return nc


def kernel(left_features, right_features, Wq, bq, Wk, bk, Wv, bv, Wo, bo, rel_emb,
           _trace=False):
    from concourse.bass_utils import run_bass_kernel_spmd

    bf = ml_dtypes.bfloat16
    f8 = ml_dtypes.float8_e4m3
    if "nc" not in _CACHE:
        _CACHE["nc"] = _build()
    nc = _CACHE["nc"]

    consts = _hostprep(Wq, bq, Wk, bk, Wv, bv, Wo, bo, rel_emb)
    lf = np.asarray(left_features, np.float32)
    rf = np.asarray(right_features, np.float32)

    def wmajor(x, dt, s):  # (C, H, W) -> (C, HW) with col = w*32 + i
        return np.ascontiguousarray(
            x.transpose(0, 2, 1).reshape(C, HW) * s).astype(dt)

    qkdt, qks = (f8, SX) if QK_FP8 else (bf, 1.0)
    vdt, vs = (f8, SX) if V_FP8 else (bf, 1.0)
    in_maps = []
    for core in range(8):
        d, b = divmod(core, 4)
        qf = lf[b] if d == 0 else rf[b]
        rfb = rf[b] if d == 0 else lf[b]
        m = dict(consts)
        m["xq"] = wmajor(qf, qkdt, qks)
        m["xr"] = wmajor(rfb, qkdt, qks)
        if V_FP8 != QK_FP8:
            m["xrv"] = wmajor(rfb, vdt, vs)
        in_maps.append(m)

    res = run_bass_kernel_spmd(nc, in_maps, list(range(8)), trace=_trace)
    _CACHE["last_result"] = res

    def unperm(o):  # [C, HW w-major] -> (C, H, W)
        return np.ascontiguousarray(o.reshape(C, W, H).transpose(0, 2, 1))

    wr = np.stack([unperm(res.results[b]["out"]) for b in range(4)])
    wl = np.stack([unperm(res.results[4 + b]["out"]) for b in range(4)])
    left_att = np.concatenate([lf, wr], axis=1)
    right_att = np.concatenate([rf, wl], axis=1)
    return (left_att, right_att)


# revision 14
# speedup vs baseline: 4.5422x; 1.5279x over previous
# Trainium2 Bass kernel for CoAttentionModule (axial co-attention, 8 heads).
#
# Sharding: data-parallel over (direction, batch) = 2 x 4 = 8 NeuronCores.
# Core c computes weighted = _coattention(qf, rf)[b].T for its (d, b) pair;
# the host concatenates [features, weighted] per direction.
#
# On-chip layout: the hw axis is w-major everywhere (col = w*32 + i, i = h
# index); the host pre-permutes features and un-permutes the output. This
# makes every matmul stationary operand a contiguous SBUF slice.
#
# The Q and K projections run in fp8e4m3 with DoubleRow perf
# mode (contracts 256 channels per pass = 2x bf16 throughput). Scales keep
# every fp8 operand inside the e4m3 normal range:
#   features x8 = fp8(x * SX), weights W8 = fp8(W * SW)
#   q_s/k_s are scaled by SX*SW; the dequant folds into the softmax scale.
#   v_s scaled by SX*SW; attT8 = av_psum * SA/(SX*SW)  (att * SA, SA=16)
#   out = psum / (SA*SW) + bias
# Attention internals (scores, softmax, AV) stay bf16/fp32 as in the bf16
# kernel:
#   qT = Wq.T @ xq (+bq)          [c_out, hw]
#   kT = Wk.T @ xr  + RWF         RWF[c,(w,k)] = rel[(k-w)%63, c]*SX*SW (rel_w
#                                 folded into keys; bk cancels in softmax)
#   v  = xr.T @ Wv                [(w,k), c]
#   QAUG[t', col(w,i)] = sum_c relroll q  (63 aug rows, rolled rel_emb.T)
#   scores tile (head n, w-group of 4) [128=(w,i), 128=(w,k)]:
#       q.k' + QAUG.KAUG(one-hot) + WIND.KMASK(-1e30 off-diag mask channels)
#   softmax: exp(scale=1/(16*(SX*SW)^2)) with accum_out row sums -> recip
#   probsT via DVE 32x32 stream transpose (block-diagonal => exact transpose)
#   avT[c,(w,i)] = v.T @ probsT ; output proj outT = Wo.T @ attT + bo'
#   (bv folded on host: bo' = bv @ Wo + bo; bk dropped: softmax-invariant)
import numpy as np
import ml_dtypes

B, C, H, W = 4, 2048, 32, 32
HW = H * W
NH, HD = 8, 256
T = 2 * max(H, W) - 1  # 63
NC = C // 128  # 16 chunks

SX = 8.0     # feature fp8 scale
SW = 512.0   # weight fp8 scale
SA = 16.0    # attT fp8 scale (att values ~N(0, 0.4^2), absmax ~5)

QK_FP8 = True
V_FP8 = False
O_FP8 = False

_CACHE = {}


def _hostprep(Wq, bq, Wk, bk, Wv, bv, Wo, bo, rel_emb):
    bf = ml_dtypes.bfloat16
    f8 = ml_dtypes.float8_e4m3
    f32 = np.float32
    Wq, Wk, Wv, Wo = (np.asarray(a, f32) for a in (Wq, Wk, Wv, Wo))
    rel = np.asarray(rel_emb, f32)  # [63, 256]
    ii = np.arange(32)
    sqs = SX * SW if QK_FP8 else 1.0  # scale of q_s and k_s

    # lhsT blobs [co, p, ci*128+m]: one contiguous [128, 2048] DMA per co chunk
    def lchunks(Wm, dt, s):
        return np.ascontiguousarray(
            (Wm * s).reshape(NC, 128, NC, 128).transpose(2, 1, 0, 3).reshape(NC, 128, C)
        ).astype(dt)

    wq_l = lchunks(Wq, f8 if QK_FP8 else bf, SW if QK_FP8 else 1.0)
    wk_l = lchunks(Wk, f8 if QK_FP8 else bf, SW if QK_FP8 else 1.0)
    wo_l = lchunks(Wo, f8 if O_FP8 else bf, SW if O_FP8 else 1.0)
    # V weights per head [n, p, ci*256+m]: one contiguous [128, 4096] DMA per head
    wv_r = np.ascontiguousarray(
        (Wv * (SW if V_FP8 else 1.0))
        .reshape(NC, 128, NH, HD).transpose(2, 1, 0, 3).reshape(NH, 128, NC * HD)
    ).astype(f8 if V_FP8 else bf)

    bq_c = np.ascontiguousarray((np.asarray(bq, f32) * sqs).reshape(NC, 128).T)
    bo2 = np.asarray(bv, f32) @ Wo + np.asarray(bo, f32)
    bo2_c = np.ascontiguousarray(bo2.reshape(NC, 128).T)  # [128,16]

    w_idx, k_idx = np.meshgrid(np.arange(32), np.arange(32), indexing="ij")
    # rel_w fold table, w-major [2, 128, 1024]: rwf[ch, p, w*32+k] = rel[(k-w)%63, ch*128+p]
    rwf = (rel * sqs)[(k_idx - w_idx) % T].reshape(HW, HD)  # [(w,k), 256]
    rwf = np.ascontiguousarray(rwf.T.reshape(2, 128, HW)).astype(bf)
    # rolled rel_emb.T for QAUG: relroll[p, (i, ch, t')] = rel[(t'-i)%63, ch*128+p]
    # t' padded 63->128 with zeros so the stationary operand is 128 wide (FWL)
    relroll = np.zeros((128, 32 * 2 * 128), f32)
    for i in range(32):
        for ch in range(2):
            blk = (rel * sqs)[(np.arange(T) - i) % T, ch * 128:(ch + 1) * 128]
            relroll[:, (i * 2 + ch) * 128:(i * 2 + ch) * 128 + T] = blk.T
    relroll = relroll.astype(bf)
    # key-side aug channels [96, 1024] w-major: rows 0:63 one-hot rel gather
    # (kaug[t, w*32+k] = t==k), row 63 zero, rows 64:96 block-diag mask
    # (kmask[w', w*32+k] = 0 if w==w' else -1e30). Query side: rows 0:63 QAUG,
    # row 63 zero, rows 64:96 w-indicator.
    kaug = np.zeros((96, HW), f32)
    kaug[k_idx.reshape(-1), np.arange(HW)] = 1.0
    kaug[64:96] = -1e30
    wind = np.zeros((32, HW), f32)
    for w in range(32):
        wind[w, w * 32 + ii] = 1.0  # query col w*32+i
        kaug[64 + w, w * 32 + ii] = 0.0  # key col w*32+k
    kaug = kaug.astype(bf)
    wind = wind.astype(bf)

    return dict(wq_l=wq_l, wk_l=wk_l, wo_l=wo_l, wv_r=wv_r, bq_c=bq_c,
                bo2_c=bo2_c, rwf=rwf, relroll=relroll, kaug=kaug, wind=wind)


def _build(timing_twin=False, loop=1):
    import concourse.bacc as bacc
    import concourse.mybir as mybir
    import concourse.tile as tile

    F32, BF16, F8 = mybir.dt.float32, mybir.dt.bfloat16, mybir.dt.float8e4
    DR = mybir.MatmulPerfMode.DoubleRow
    QKDT = F8 if QK_FP8 else BF16
    VDT = F8 if V_FP8 else BF16
    ODT = F8 if O_FP8 else BF16
    sqs = SX * SW if QK_FP8 else 1.0
    svs = SX * SW if V_FP8 else 1.0
    EXPSCALE = float(1.0 / (16.0 * sqs * sqs))
    ATTSCALE = float((SA if O_FP8 else 1.0) / svs)
    ODEQ = float(1.0 / (SA * SW)) if O_FP8 else 1.0
    nc = bacc.Bacc(None, target_bir_lowering=False)

    if timing_twin:
        # timing-equivalent NEFF: big tensors live in internal DRAM scratch
        # (no per-call host staging), only a tiny external in/out pair.
        def declare(name, shape, dt, isOutput=False):
            return nc.dram_tensor(name, shape, dt)
        tiny_in = nc.declare_dram_parameter("tiny_in", [1, 4], F32, isOutput=False)
        tiny_out = nc.declare_dram_parameter("tiny_out", [1, 4], F32, isOutput=True)
    else:
        declare = nc.declare_dram_parameter

    xq = declare("xq", [C, HW], QKDT, isOutput=False)
    xr = declare("xr", [C, HW], QKDT, isOutput=False)
    xrv = declare("xrv", [C, HW], VDT, isOutput=False) if V_FP8 != QK_FP8 else None
    wq_l = declare("wq_l", [NC, 128, C], QKDT, isOutput=False)
    wk_l = declare("wk_l", [NC, 128, C], QKDT, isOutput=False)
    wo_l = declare("wo_l", [NC, 128, C], ODT, isOutput=False)
    wv_r = declare("wv_r", [NH, 128, NC * HD], VDT, isOutput=False)
    bq_c = declare("bq_c", [128, NC], F32, isOutput=False)
    bo2_c = declare("bo2_c", [128, NC], F32, isOutput=False)
    rwf = declare("rwf", [2, 128, HW], BF16, isOutput=False)
    relroll = declare("relroll", [128, 32 * 2 * 128], BF16, isOutput=False)
    kaug = declare("kaug", [96, HW], BF16, isOutput=False)
    wind = declare("wind", [32, HW], BF16, isOutput=False)
    out = declare("out", [C, HW], F32, isOutput=True)

    EXP = mybir.ActivationFunctionType.Exp

    with tile.TileContext(nc) as tc:
        with (
            tc.tile_pool(name="feat", bufs=2) as feat_pool,
            tc.tile_pool(name="att", bufs=1) as att_pool,
            tc.tile_pool(name="const", bufs=1) as const_pool,
            tc.tile_pool(name="head", bufs=2) as head_pool,
            tc.tile_pool(name="wstr", bufs=4) as wstr_pool,
            tc.tile_pool(name="probs", bufs=5) as probs_pool,
            tc.tile_pool(name="outs", bufs=3) as outs_pool,
            tc.tile_pool(name="psum", bufs=4, space="PSUM") as psum_pool,
            tc.tile_pool(name="psumb", bufs=3, space="PSUM") as psumb_pool,
            tc.tile_pool(name="psumq", bufs=1, space="PSUM") as psumq_pool,
        ):
            # ---- load features + constants (resident) ----
            # xq first (gates the very first Q-proj groups), then xr, then
            # constants (needed later) so the PE cold-start wait is minimal.
            xqt = feat_pool.tile([128, NC * HW], QKDT, tag="feat")
            xrt = feat_pool.tile([128, NC * HW], QKDT, tag="feat")
            for cc in range(NC):
                nc.sync.dma_start(xqt[:, cc * HW:(cc + 1) * HW], xq[cc * 128:(cc + 1) * 128, :])
            for cc in range(NC):
                nc.sync.dma_start(xrt[:, cc * HW:(cc + 1) * HW], xr[cc * 128:(cc + 1) * 128, :])
            if xrv is not None:
                xrvt = feat_pool.tile([128, NC * HW], VDT, tag="featv", bufs=1)
                for cc in range(NC):
                    nc.sync.dma_start(xrvt[:, cc * HW:(cc + 1) * HW], xrv[cc * 128:(cc + 1) * 128, :])
            else:
                xrvt = xrt
            attT = att_pool.tile([128, NC * HW], ODT)

            c_kaug = const_pool.tile([96, HW], BF16)
            nc.sync.dma_start(c_kaug[:], kaug[:])
            c_wind = const_pool.tile([32, HW], BF16)
            nc.sync.dma_start(c_wind[:], wind[:])
            c_rwf = const_pool.tile([128, 2 * HW], BF16)
            nc.sync.dma_start(c_rwf[:, 0:HW], rwf[0])
            nc.sync.dma_start(c_rwf[:, HW:2 * HW], rwf[1])
            c_roll = const_pool.tile([128, 32 * 2 * 128], BF16)
            nc.sync.dma_start(c_roll[:], relroll[:])
            c_bq = const_pool.tile([128, NC], F32)
            nc.sync.dma_start(c_bq[:], bq_c[:])
            c_bo = const_pool.tile([128, NC], F32)
            nc.sync.dma_start(c_bo[:], bo2_c[:])

            x3q = xqt.rearrange("p (c w) -> p c w", c=NC)
            x3r = xrt.rearrange("p (c w) -> p c w", c=NC)
            x3rv = xrvt.rearrange("p (c w) -> p c w", c=NC)
            att3 = attT.rearrange("p (c w) -> p c w", c=NC)

            for rep in range(loop):
                for n in range(NH):
                    sq = head_pool.tile([128, 2 * HW], BF16, tag="sq")
                    sk = head_pool.tile([128, 2 * HW], BF16, tag="sk")
                    sv = head_pool.tile([128, NH * HD], BF16, tag="sv")
                    sqa = head_pool.tile([96, HW], BF16, tag="sqa")
                    swv = head_pool.tile([128, NC * HD], VDT, tag="swv")

                    # stage this head's V weights (4 parallel chunk DMAs)
                    for j in range(4):
                        nc.sync.dma_start(swv[:, j * NC * HD // 4:(j + 1) * NC * HD // 4],
                                          wv_r[n][:, j * NC * HD // 4:(j + 1) * NC * HD // 4])
                    # aug rows 32:96: zero row 63, w-indicator rows 64:96
                    nc.vector.memset(sqa[:], 0.0)
                    nc.vector.tensor_copy(sqa[64:96, :], c_wind[:])

                    # ---- Q / K projections: psum[co2, cols] = W.T @ x ----
                    for which in range(2):  # 0 = Q, 1 = K
                        wsrc = wq_l if which == 0 else wk_l
                        xsrc = x3q if which == 0 else x3r
                        dst = sq if which == 0 else sk
                        for co2 in range(2):
                            co = n * 2 + co2
                            wt = wstr_pool.tile([128, C], QKDT, tag="wl")
                            for j in range(4):
                                nc.sync.dma_start(wt[:, j * 512:(j + 1) * 512],
                                                  wsrc[co][:, j * 512:(j + 1) * 512])
                            w3 = wt.rearrange("p (c m) -> p c m", c=NC)
                            if QK_FP8:
                                for g in range(4):
                                    ps = psum_pool.tile([128, 256], F32, tag="pp")
                                    for c2 in range(8):
                                        nc.tensor.matmul(
                                            ps[:], w3[:, 2 * c2:2 * c2 + 2, :],
                                            xsrc[:, 2 * c2:2 * c2 + 2, g * 256:(g + 1) * 256],
                                            start=(c2 == 0), stop=(c2 == 7),
                                            perf_mode=DR)
                                    dpos = dst[:, co2 * HW + g * 256: co2 * HW + (g + 1) * 256]
                                    if which == 0:
                                        nc.vector.tensor_scalar_add(dpos, ps[:], c_bq[:, co:co + 1])
                                    else:
                                        nc.vector.tensor_add(
                                            dpos, ps[:],
                                            c_rwf[:, co2 * HW + g * 256: co2 * HW + (g + 1) * 256])
                            else:
                                for h2 in range(2):
                                    ps = psum_pool.tile([128, 512], F32, tag="pp")
                                    for ci in range(NC):
                                        nc.tensor.matmul(
                                            ps[:], w3[:, ci, :],
                                            xsrc[:, ci, h2 * 512:(h2 + 1) * 512],
                                            start=(ci == 0), stop=(ci == NC - 1))
                                    dpos = dst[:, co2 * HW + h2 * 512: co2 * HW + (h2 + 1) * 512]
                                    if which == 0:
                                        nc.vector.tensor_scalar_add(dpos, ps[:], c_bq[:, co:co + 1])
                                    else:
                                        nc.vector.tensor_add(
                                            dpos, ps[:],
                                            c_rwf[:, co2 * HW + h2 * 512: co2 * HW + (h2 + 1) * 512])

                    # ---- V projection, w-major rows ----
                    w3v = swv.rearrange("p (c m) -> p c m", c=NC)
                    for wg in range(8):
                        psv = psum_pool.tile([128, 256], F32, tag="pp")
                        if V_FP8:
                            for c2 in range(8):
                                nc.tensor.matmul(
                                    psv[:],
                                    x3rv[:, 2 * c2:2 * c2 + 2, wg * 128:(wg + 1) * 128],
                                    w3v[:, 2 * c2:2 * c2 + 2, :],
                                    start=(c2 == 0), stop=(c2 == 7),
                                    perf_mode=DR)
                        else:
                            for ci in range(NC):
                                nc.tensor.matmul(
                                    psv[:], x3rv[:, ci, wg * 128:(wg + 1) * 128],
                                    w3v[:, ci, :],
                                    start=(ci == 0), stop=(ci == NC - 1))
                        nc.vector.tensor_copy(sv[:, wg * HD:(wg + 1) * HD], psv[:])

                    # ---- QAUG: per query-row i, rolled rel_emb.T contraction ----
                    for half in range(2):
                        pqa = psumq_pool.tile([128, 512], F32, tag="qa")
                        for io in range(16):
                            i = half * 16 + io
                            for ch in range(2):
                                nc.tensor.matmul(
                                    pqa[:, io * 32:(io + 1) * 32],
                                    c_roll[:, (i * 2 + ch) * 128:(i * 2 + ch + 1) * 128],
                                    sq[:, ch * HW + i: (ch + 1) * HW: 32],
                                    start=(ch == 0), stop=(ch == 1))
                        # pqa cols are (i, w) pairs; sqa is w-major (w*32+i)
                        nc.vector.tensor_copy(
                            sqa[0:T, :].rearrange("p (w i) -> p i w", i=32)[:, half * 16:(half + 1) * 16, :],
                            pqa[0:T, :].rearrange("p (i w) -> p i w", w=32))

                    # ---- attention per w-group ----
                    for wg in range(8):
                        sc = psumb_pool.tile([128, 128], F32, tag="sa")
                        nc.tensor.matmul(sc[:], sq[:, wg * 128:(wg + 1) * 128],
                                         sk[:, wg * 128:(wg + 1) * 128],
                                         start=True, stop=False)
                        nc.tensor.matmul(sc[:], sq[:, HW + wg * 128: HW + (wg + 1) * 128],
                                         sk[:, HW + wg * 128: HW + (wg + 1) * 128],
                                         start=False, stop=False)
                        nc.tensor.matmul(sc[:], sqa[:, wg * 128:(wg + 1) * 128],
                                         c_kaug[:, wg * 128:(wg + 1) * 128],
                                         start=False, stop=True)
                        probs = probs_pool.tile([128, 128], BF16, tag="pr")
                        sums = probs_pool.tile([128, 1], F32, tag="sm")
                        recip = probs_pool.tile([128, 1], F32, tag="rc")
                        nc.scalar.activation(probs[:], sc[:], EXP, scale=EXPSCALE,
                                             accum_out=sums[:])
                        nc.vector.reciprocal(recip[:], sums[:])
                        nc.vector.tensor_scalar_mul(probs[:], probs[:], recip[:])
                        probsT = probs_pool.tile([128, 128], BF16, tag="prT")
                        nc.vector.transpose(probsT[:], probs[:])
                        for ch in range(2):
                            av = psumb_pool.tile([128, 128], F32, tag="sa")
                            nc.tensor.matmul(
                                av[:], sv[:, wg * HD + ch * 128: wg * HD + (ch + 1) * 128],
                                probsT[:], start=True, stop=True)
                            apos = attT[:, (n * 2 + ch) * HW + wg * 128:
                                        (n * 2 + ch) * HW + (wg + 1) * 128]
                            if ATTSCALE == 1.0:
                                nc.vector.tensor_copy(apos, av[:])
                            else:
                                nc.vector.tensor_scalar_mul(apos, av[:], ATTSCALE)

                # ---- output projection ----
                for co in range(NC):
                    wt = wstr_pool.tile([128, C], ODT, tag="wo")
                    for j in range(4):
                        nc.sync.dma_start(wt[:, j * 512:(j + 1) * 512],
                                          wo_l[co][:, j * 512:(j + 1) * 512])
                    w3 = wt.rearrange("p (c m) -> p c m", c=NC)
                    if O_FP8:
                        for g in range(4):
                            ps = psum_pool.tile([128, 256], F32, tag="pp")
                            for c2 in range(8):
                                nc.tensor.matmul(
                                    ps[:], w3[:, 2 * c2:2 * c2 + 2, :],
                                    att3[:, 2 * c2:2 * c2 + 2, g * 256:(g + 1) * 256],
                                    start=(c2 == 0), stop=(c2 == 7),
                                    perf_mode=DR)
                            ot = outs_pool.tile([128, 256], F32, tag="ot")
                            nc.vector.tensor_scalar(
                                ot[:], ps[:], ODEQ, c_bo[:, co:co + 1],
                                mybir.AluOpType.mult, mybir.AluOpType.add)
                            nc.sync.dma_start(
                                out[co * 128:(co + 1) * 128, g * 256:(g + 1) * 256], ot[:])
                    else:
                        for h2 in range(2):
                            ps = psum_pool.tile([128, 512], F32, tag="pp")
                            for ci in range(NC):
                                nc.tensor.matmul(
                                    ps[:], w3[:, ci, :],
                                    att3[:, ci, h2 * 512:(h2 + 1) * 512],
                                    start=(ci == 0), stop=(ci == NC - 1))
                            ot = outs_pool.tile([128, 512], F32, tag="ot")
                            nc.vector.tensor_scalar_add(ot[:], ps[:], c_bo[:, co:co + 1])
                            nc.sync.dma_start(
                                out[co * 128:(co + 1) * 128, h2 * 512:(h2 + 1) * 512], ot[:])

                if timing_twin:
                    tt = outs_pool.tile([1, 4], F32, tag="tt")
                    nc.sync.dma_start(tt[:], tiny_in[:])
                    nc.sync.dma_start(tiny_out[:], tt[:])

            if timing_twin:
                tt = outs_pool.tile([1, 4], F32, tag="tt")
                nc.sync.dma_start(tt[:], tiny_in[:])
                nc.sync.dma_start(tiny_out[:], tt[:])

    nc.finalize()
    return nc


def kernel(left_features, right_features, Wq, bq, Wk, bk, Wv, bv, Wo, bo, rel_emb,
           _trace=False):
    from concourse.bass_utils import run_bass_kernel_spmd

    bf = ml_dtypes.bfloat16
    f8 = ml_dtypes.float8_e4m3
    if "nc" not in _CACHE:
        _CACHE["nc"] = _build()
    nc = _CACHE["nc"]

    consts = _hostprep(Wq, bq, Wk, bk, Wv, bv, Wo, bo, rel_emb)
    lf = np.asarray(left_features, np.float32)
    rf = np.asarray(right_features, np.float32)

    def wmajor(x, dt, s):  # (C, H, W) -> (C, HW) with col = w*32 + i
        return np.ascontiguousarray(
            x.transpose(0, 2, 1).reshape(C, HW) * s).astype(dt)

    qkdt, qks = (f8, SX) if QK_FP8 else (bf, 1.0)
    vdt, vs = (f8, SX) if V_FP8 else (bf, 1.0)
    in_maps = []
    for core in range(8):
        d, b = divmod(core, 4)
        qf = lf[b] if d == 0 else rf[b]
        rfb = rf[b] if d == 0 else lf[b]
        m = dict(consts)
        m["xq"] = wmajor(qf, qkdt, qks)
        m["xr"] = wmajor(rfb, qkdt, qks)
        if V_FP8 != QK_FP8:
            m["xrv"] = wmajor(rfb, vdt, vs)
        in_maps.append(m)

    res = run_bass_kernel_spmd(nc, in_maps, list(range(8)), trace=_trace)
    _CACHE["last_result"] = res

    def unperm(o):  # [C, HW w-major] -> (C, H, W)
        return np.ascontiguousarray(o.reshape(C, W, H).transpose(0, 2, 1))

    wr = np.stack([unperm(res.results[b]["out"]) for b in range(4)])
    wl = np.stack([unperm(res.results[4 + b]["out"]) for b in range(4)])
    left_att = np.concatenate([lf, wr], axis=1)
    right_att = np.concatenate([rf, wl], axis=1)
    return (left_att, right_att)
